# revision 16
# baseline (speedup 1.0000x reference)
"""Causal multi-head attention kernel for Trainium2 (8 NeuronCores).

Problem: x[1,2048,1024] -> qkv proj (W_qkv[1024,3072]) -> 64 heads of dim 16
         -> causal softmax attention -> out proj (W_out[1024,1024]).

Sharding: Megatron-style head parallelism. Each of the 8 cores owns 8 heads
(a 128-wide column slice of W_qkv per q/k/v and a 128-row slice of W_out),
computes a partial output projection, and the host sums the 8 partials
(the "all-reduce").

v2 pipeline notes (vs the phase-separated v1):
  * ScalarE exp over ~19M score elements is the critical resource (~160us),
    so the whole kernel is restructured as one ACT-paced stream:
    per query block qn: qkv(qn) -> attention tiles (kc-outer, a-inner) with
    qkv(qn+1) + outproj(qn-1) matmul units interleaved as PE filler inside
    the ACT-bound stretches, x-prefetch(qn+1) on DMA/GpSimd meanwhile.
  * ~1/3 of the full (non-diagonal) exp tiles are computed on the otherwise
    co-idle VectorE via a Schraudolph-style bf16 exp2 bit trick:
    bf16_bits(2^t) ~= round(128*t + 16256 - sigma); one tensor_scalar
    (mult+add, fp32 PSUM in -> int16 out) then bitcast int16->bf16.
    Error is ~3% on those p values only; with ~1/5 of all tiles tricked the
    final output error stays ~0.5%, well under the 2e-2 gate.
  * Diagonal tiles compute/exp/accumulate only the valid causal suffix
    (no dead-prefix memsets); the 128-wide diagonal stripe is tri-masked.
  * Softmax 1/rowsum via 2-pass Ln then Exp(scale=-1) (Square pass dropped:
    NaNs in never-read lanes are harmless; rowsum lanes are positive).
  * PSUM budget: 2x2-bank score sets + 2x1-bank PV accumulators +
    2x1-bank filler accumulators (qkv/outproj) = 8 banks.

Self-contained: hardcodes all shapes; host code only slices inputs per core
and sums the 8 partial outputs.
"""

import numpy as np
from contextlib import ExitStack

import ml_dtypes

import concourse.bass as bass
import concourse.tile as tile
from concourse import mybir
from concourse.bass_utils import run_bass_kernel_spmd

F32 = mybir.dt.float32
BF16 = mybir.dt.bfloat16
I16 = mybir.dt.int16
AF = mybir.ActivationFunctionType

T = 2048
C = 1024
HDIM = 16
NHEADS = 64
NCORES = 8
HPC = NHEADS // NCORES      # 8 heads per core
CSLICE = HPC * HDIM         # 128 channel slice per core
G = 2                       # head groups of 4 per core
NCH = C // 128              # 8 contraction chunks
NT = T // 128               # 16 token chunks of 128
NQ = T // 512               # 4 query blocks of 512

# Schraudolph bf16 exp2: bits = round(EXPQ_MUL * s + EXPQ_ADD) viewed as bf16
# approximates exp(0.25*s).  128*log2(e)*0.25 = 46.166...; 16256 = 127<<7.
EXPQ_MUL = 128.0 * 0.25 * 1.4426950408889634
EXPQ_ADD = 16256.0 - 5.5
TRICK_EVERY = 3            # every 3rd full tile -> DVE trick exp

_CACHE = {}


def _legalize_waits(nc):
    """This neuronxcc/walrus build encodes at most ONE sync-wait per
    instruction (two on EventSemaphore) — multi-wait sync_info dies in
    codegen with "Too many sync wait commands".  Hoist excess waits into
    standalone EventSemaphore instructions on the same engine immediately
    before the instruction (engine queues are in-order, so semantics are
    preserved)."""
    import bass_rust
    n = 0
    for f in nc.m.functions:
        for blk in f.blocks:
            out = []
            changed = False
            for inst in blk.instructions:
                si = inst.sync_info
                waits = list(si.on_wait) if si is not None and si.on_wait else []
                cap = 2 if isinstance(inst, mybir.InstEventSemaphore) else 1
                if len(waits) > cap:
                    extra, keep = waits[:-cap], waits[-cap:]
                    for i in range(0, len(extra), 2):
                        ev = mybir.InstEventSemaphore(
                            name=f"evwait-{n}", ins=[], outs=[])
                        n += 1
                        ev.engine = inst.engine
                        ev.sync_info = bass_rust.SyncInfo(
                            on_wait=extra[i:i + 2], on_update=[])
                        out.append(ev)
                    inst.sync_info = bass_rust.SyncInfo(
                        on_wait=keep,
                        on_update=list(si.on_update) if si.on_update else [])
                    changed = True
                out.append(inst)
            if changed:
                blk.instructions = out
    return n


def _build_nc():
    nc = bass.Bass()

    x_d = nc.declare_dram_parameter("x", [T, C], F32, isOutput=False)
    wq_d = nc.declare_dram_parameter("wq", [C, CSLICE], F32, isOutput=False)
    wk_d = nc.declare_dram_parameter("wk", [C, CSLICE], F32, isOutput=False)
    wv_d = nc.declare_dram_parameter("wv", [C, CSLICE], F32, isOutput=False)
    wo_d = nc.declare_dram_parameter("wo", [G * 128, C], F32, isOutput=False)
    bq_d = nc.declare_dram_parameter("bq", [G, 128], F32, isOutput=False)
    bk_d = nc.declare_dram_parameter("bk", [G, 128], F32, isOutput=False)
    bv_d = nc.declare_dram_parameter("bv", [1, CSLICE], F32, isOutput=False)
    bo_d = nc.declare_dram_parameter("bo", [1, C], F32, isOutput=False)
    tri_d = nc.declare_dram_parameter("tri", [128, 128], BF16, isOutput=False)
    y_d = nc.declare_dram_parameter("y", [T, C], F32, isOutput=True)

    with tile.TileContext(nc) as tc, ExitStack() as ctx:
        consts = ctx.enter_context(tc.tile_pool(name="consts", bufs=1))
        stage = ctx.enter_context(tc.tile_pool(name="stage", bufs=3))
        epool = ctx.enter_context(tc.tile_pool(name="epool", bufs=6))
        small = ctx.enter_context(tc.tile_pool(name="small", bufs=2))

        psco = ctx.enter_context(tc.tile_pool(name="psco", bufs=2, space="PSUM"))
        ppv = ctx.enter_context(tc.tile_pool(name="ppv", bufs=1, space="PSUM"))
        pfil = ctx.enter_context(tc.tile_pool(name="pfil", bufs=2, space="PSUM"))

        # ---- constants ----
        tri = consts.tile([128, 128], BF16)
        nc.sync.dma_start(out=tri, in_=tri_d[:, :])
        eps_sb = consts.tile([128, 1], F32)
        nc.vector.memset(eps_sb, 1e-30)
        bq_sb = consts.tile([128, G], F32)
        nc.sync.dma_start(out=bq_sb, in_=bq_d.rearrange("g p -> p g"))
        bk_sb = consts.tile([128, G], F32)
        nc.sync.dma_start(out=bk_sb, in_=bk_d.rearrange("g p -> p g"))
        # free-dim-varying biases must be physically replicated across
        # partitions (DVE operands need nonzero partition step)
        bv_sb = consts.tile([128, CSLICE], F32)
        nc.sync.dma_start(out=bv_sb, in_=bv_d[0:1, :].to_broadcast((128, CSLICE)))
        bo_sb = consts.tile([128, C], F32)
        nc.sync.dma_start(out=bo_sb, in_=bo_d[0:1, :].to_broadcast((128, C)))

        # ---- weights: load fp32, cast to bf16 stationaries ----
        # wq/wk spaced: per group g, chunk cc: [128c, 128] with head j's 16
        # cols at free offset 32j.
        wq_sb = consts.tile([128, G, NCH, 128], BF16)
        wk_sb = consts.tile([128, G, NCH, 128], BF16)
        nc.vector.memset(wq_sb, 0.0)
        nc.vector.memset(wk_sb, 0.0)
        wv_sb = consts.tile([128, NCH, CSLICE], BF16)
        wo_sb = consts.tile([128, G, C], BF16)
        def load_weights():
            wqf = stage.tile([128, NCH, CSLICE], F32, tag="wstage", name="wqf", bufs=1)
            nc.sync.dma_start(out=wqf, in_=wq_d.rearrange("(a p) w -> p a w", p=128))
            wkf = stage.tile([128, NCH, CSLICE], F32, tag="wstage2", name="wkf", bufs=1)
            nc.sync.dma_start(out=wkf, in_=wk_d.rearrange("(a p) w -> p a w", p=128))
            wvf = stage.tile([128, NCH, CSLICE], F32, tag="wstage3", name="wvf", bufs=1)
            nc.sync.dma_start(out=wvf, in_=wv_d.rearrange("(a p) w -> p a w", p=128))
            for g in range(G):
                for j in range(4):
                    h = 4 * g + j
                    nc.vector.tensor_copy(
                        wq_sb[:, g, :, 32 * j:32 * j + HDIM],
                        wqf[:, :, HDIM * h:HDIM * (h + 1)])
                    nc.vector.tensor_copy(
                        wk_sb[:, g, :, 32 * j:32 * j + HDIM],
                        wkf[:, :, HDIM * h:HDIM * (h + 1)])
            nc.vector.tensor_copy(wv_sb, wvf)
            for g in range(G):
                wof = stage.tile([128, C], F32, tag="wofull", name="wof", bufs=2)
                nc.sync.dma_start(out=wof, in_=wo_d[g * 128:(g + 1) * 128, :])
                nc.vector.tensor_copy(wo_sb[:, g, :], wof)

        # ---- persistent activations ----
        xT = consts.tile([128, NCH, T], BF16)   # xT[c, cc, t] = x[t, 128cc+c]
        qT = consts.tile([128, G, T], BF16)     # spaced: head j at part 32j
        kT = consts.tile([128, G, T], BF16)
        V = consts.tile([128, NT, HPC * 32], BF16)  # [t, tt, 8*32]: 16 dims +
        # rowsum-ones col + zero pad per head (packed PV writes 32 rows/head)
        vr = V.rearrange("p t (h e) -> p t h e", h=HPC)
        nc.vector.memset(vr[:, :, :, HDIM:32], 0.0)
        nc.vector.memset(vr[:, :, :, HDIM:HDIM + 1], 1.0)
        # group-spaced attn output: head j of group g at partitions
        # 32j..32j+15 of attnT[:, g, :]; rows 16..31 of each quadrant must be
        # ZERO (wo has zero rows there, but 0*garbage-NaN would poison).
        attnT = consts.tile([128, G, T], BF16)
        nc.vector.memset(attnT, 0.0)

        # ---- pipeline building blocks ----
        def x_tile(tt, on_act):
            xs = stage.tile([128, C], F32, tag="xload", name="xs", bufs=4)
            nc.sync.dma_start(out=xs, in_=x_d[tt * 128:(tt + 1) * 128, :])
            xb = stage.tile([128, C], BF16, tag="xcast", name="xb", bufs=4)
            if on_act:
                nc.scalar.activation(out=xb, in_=xs, func=AF.Copy)
            else:
                nc.vector.tensor_copy(xb, xs)
            nc.sync.dma_start_transpose(
                out=xT[:, :, tt * 128:(tt + 1) * 128], in_=xb)

        qk_open = {}

        def qk_half(g, qn, which, half):
            """One 4-chunk half of a q or k projection accumulation.
            half=0 opens the PSUM group; half=1 finishes it + bias."""
            w_sb, b_sb, dst = ((wq_sb, bq_sb, qT) if which == "q"
                               else (wk_sb, bk_sb, kT))
            if half == 0:
                qk_open[(which, g)] = pfil.tile(
                    [128, 512], F32, tag="fil", name="filps")
            ps_t = qk_open[(which, g)]
            for i in range(4):
                cc = 4 * half + i
                nc.tensor.matmul(
                    out=ps_t, lhsT=w_sb[:, g, cc, :],
                    rhs=xT[:, cc, qn * 512:(qn + 1) * 512],
                    start=(cc == 0), stop=(cc == NCH - 1),
                )
            if half == 1:
                del qk_open[(which, g)]
                nc.vector.tensor_scalar_add(
                    out=dst[:, g, qn * 512:(qn + 1) * 512], in0=ps_t,
                    scalar1=b_sb[:, g:g + 1],
                )

        def v_tile(tt):
            ps_t = pfil.tile([128, 512], F32, tag="fil", name="vps")
            ps = ps_t[:, 0:CSLICE]
            for cc in range(NCH):
                nc.tensor.matmul(
                    out=ps, lhsT=xT[:, cc, tt * 128:(tt + 1) * 128],
                    rhs=wv_sb[:, cc, :],
                    start=(cc == 0), stop=(cc == NCH - 1),
                )
            nc.vector.tensor_tensor(
                vr[:, tt, :, 0:HDIM], ps.rearrange("p (h e) -> p h e", h=HPC),
                bv_sb.rearrange("p (h e) -> p h e", h=HPC),
                mybir.AluOpType.add,
            )

        def outproj_unit(tt, nn):
            ps = pfil.tile([128, 512], F32, tag="fil", name="ops")
            for g in range(G):
                nc.tensor.matmul(
                    out=ps, lhsT=attnT[:, g, tt * 128:(tt + 1) * 128],
                    rhs=wo_sb[:, g, nn * 512:(nn + 1) * 512],
                    start=(g == 0), stop=(g == G - 1),
                )
            ys = stage.tile([128, 512], F32, tag="yout", name="ys")
            nc.vector.tensor_tensor(
                ys, ps, bo_sb[:, nn * 512:(nn + 1) * 512],
                mybir.AluOpType.add,
            )
            nc.sync.dma_start(
                out=y_d[tt * 128:(tt + 1) * 128, nn * 512:(nn + 1) * 512],
                in_=ys,
            )

        # filler scheduling: closures popped between attention tile groups
        fillers = []

        def pop_fillers(k):
            for _ in range(min(k, len(fillers))):
                fillers.pop(0)()

        trick_ctr = [0]

        def attn_group(qn):
            """Attention for all 8 heads (both groups) x 512 queries (block
            qn).  Software-pipelined: scores for tile i+1 are emitted between
            exp(i) and PV(i) so ScalarE never waits on the PE queue.  Both
            groups share one 2-bank pv tile (g=0 cols 0:512, g=1 cols
            512:1024 -> independent accumulation groups per bank) so the
            normalize chain runs once per qn."""
            pv = ppv.tile([128, 1024], F32, tag="pv")
            nkc = 4 * qn + 4
            tiles = [(g, kc, a) for g in range(G)
                     for kc in range(nkc) for a in range(2)]
            ssets = {}

            def emit_S(idx):
                g, kc, a = tiles[idx]
                f0 = max(0, 128 * (kc - 4 * qn))
                sset = psco.tile([128, 1024], F32, tag="sset", name="sset")
                for jj in range(2):
                    j = 2 * a + jj
                    nc.tensor.matmul(
                        out=sset[:, 512 * jj + f0:512 * jj + 512],
                        lhsT=kT[32 * j:32 * j + HDIM, g, kc * 128:(kc + 1) * 128],
                        rhs=qT[32 * j:32 * j + HDIM, g, qn * 512 + f0:(qn + 1) * 512],
                        start=True, stop=True,
                        tile_position=(32 * j, 0),
                    )
                ssets[idx] = sset

            emit_S(0)
            for idx, (g, kc, a) in enumerate(tiles):
                jjj = kc - 4 * qn          # >=0: diagonal-straddling tile
                f0 = max(0, 128 * jjj)
                sset = ssets.pop(idx)
                et = epool.tile([128, 1024], BF16, tag="expT", name="et")
                er = et.rearrange("p (h q) -> p h q", h=2)
                sr = sset.rearrange("p (h q) -> p h q", h=2)
                if jjj >= 0:
                    # diagonal tile: exp only the valid suffix
                    nc.scalar.activation(
                        out=er[:, :, f0:512], in_=sr[:, :, f0:512],
                        func=AF.Exp, scale=0.25)
                else:
                    trick_ctr[0] += 1
                    if trick_ctr[0] % TRICK_EVERY == 0:
                        # VectorE Schraudolph bf16 exp2 bit trick
                        nc.vector.tensor_scalar(
                            out=et.bitcast(I16), in0=sset,
                            scalar1=EXPQ_MUL, scalar2=EXPQ_ADD,
                            op0=mybir.AluOpType.mult,
                            op1=mybir.AluOpType.add,
                        )
                    else:
                        nc.scalar.activation(
                            out=et, in_=sset, func=AF.Exp, scale=0.25)
                if idx + 1 < len(tiles):
                    emit_S(idx + 1)
                # one filler unit lands where PE would idle awaiting exp
                pop_fillers(1)
                if jjj >= 0:
                    # triangle-mask the diagonal stripe on (idle) GpSimd
                    nc.gpsimd.tensor_tensor(
                        er[:, :, f0:f0 + 128], er[:, :, f0:f0 + 128],
                        tri[:, None, :].to_broadcast((128, 2, 128)),
                        mybir.AluOpType.mult,
                    )
                for jj in range(2):
                    j = 2 * a + jj
                    h = 4 * g + j
                    nc.tensor.matmul(
                        out=pv[32 * j:32 * j + 32, 512 * g + f0:512 * g + 512],
                        lhsT=V[:, kc, 32 * h:32 * h + 32],
                        rhs=et[:, 512 * jj + f0:512 * jj + 512],
                        start=(kc == 0), stop=(kc == nkc - 1),
                        tile_position=(0, 32 * j),
                        # sim group tracker is partition-base blind;
                        # packed heads write disjoint partitions
                        skip_group_check=True,
                    )
            # normalize: 1/rowsum via exp(-ln(x+eps)); garbage lanes may go
            # NaN/inf but only the (positive) rowsum rows are ever read.
            ln_t = small.tile([128, 1024], F32, tag="lnt")
            nc.scalar.activation(out=ln_t, in_=pv, func=AF.Ln, bias=eps_sb[:, 0:1])
            rec_t = small.tile([128, 1024], F32, tag="rect")
            nc.scalar.activation(out=rec_t, in_=ln_t, func=AF.Exp, scale=-1.0)
            rec_rep = small.tile([128, 1024], F32, tag="recrep")
            nc.vector.stream_shuffle(rec_rep, rec_t, [HDIM] * 32)
            for j in range(4):
                nc.vector.tensor_tensor(
                    attnT[32 * j:32 * j + HDIM, :, qn * 512:(qn + 1) * 512],
                    pv[32 * j:32 * j + HDIM, :].rearrange("p (g q) -> p g q", g=2),
                    rec_rep[32 * j:32 * j + HDIM, :].rearrange("p (g q) -> p g q", g=2),
                    mybir.AluOpType.mult,
                )

        # ---- emission: fused qn-major pipeline ----
        # x tiles 0-3: loads + ACT casts first in the queues, then the big
        # weight DMAs, then the xbar transposes (so nothing head-blocks).
        xb0 = []
        for tt in range(4):
            xs = stage.tile([128, C], F32, tag="xload", name="xs", bufs=4)
            nc.sync.dma_start(out=xs, in_=x_d[tt * 128:(tt + 1) * 128, :])
            xb = stage.tile([128, C], BF16, tag="xcast", name="xb", bufs=4)
            nc.scalar.activation(out=xb, in_=xs, func=AF.Copy)
            xb0.append(xb)
        load_weights()
        for tt in range(4):
            nc.sync.dma_start_transpose(
                out=xT[:, :, tt * 128:(tt + 1) * 128], in_=xb0[tt])
        # qkv for qn=0 emitted directly (nothing to hide it under yet)
        for g in range(G):
            for half in range(2):
                qk_half(g, 0, "q", half)
            for half in range(2):
                qk_half(g, 0, "k", half)
        for tt in range(4):
            v_tile(tt)

        for qn in range(NQ):
            # stage filler for the NEXT qn's qkv + previous qn's outproj;
            # x-prefetch DMAs issue now (DMA/DVE run independently).
            if qn + 1 < NQ:
                for tt in range(4 * qn + 4, 4 * qn + 8):
                    x_tile(tt, on_act=False)
                for g in range(G):
                    for half in range(2):
                        fillers.append(
                            lambda g=g, qn=qn, h=half: qk_half(g, qn + 1, "q", h))
                    for half in range(2):
                        fillers.append(
                            lambda g=g, qn=qn, h=half: qk_half(g, qn + 1, "k", h))
                for tt in range(4 * qn + 4, 4 * qn + 8):
                    fillers.append(lambda tt=tt: v_tile(tt))
            if qn > 0:
                for tt in range(4 * (qn - 1), 4 * qn):
                    for nn in range(2):
                        fillers.append(lambda tt=tt, nn=nn: outproj_unit(tt, nn))
            attn_group(qn)
            # drain any leftover filler before moving on
            pop_fillers(len(fillers))
        for tt in range(12, 16):
            for nn in range(2):
                outproj_unit(tt, nn)
    return nc


def _make_in_maps(x, W_qkv, b_qkv, W_out, b_out):
    x2 = np.ascontiguousarray(np.asarray(x, dtype=np.float32).reshape(T, C))
    W_qkv = np.asarray(W_qkv, dtype=np.float32)
    b_qkv = np.asarray(b_qkv, dtype=np.float32)
    W_out = np.asarray(W_out, dtype=np.float32)
    b_out = np.asarray(b_out, dtype=np.float32)

    tri = np.zeros((128, 128), dtype=np.float32)
    for p in range(128):
        tri[p, p:] = 1.0
    tri = tri.astype(ml_dtypes.bfloat16)

    in_maps = []
    for p in range(NCORES):
        c0 = p * CSLICE
        wq = np.ascontiguousarray(W_qkv[:, c0:c0 + CSLICE])
        wk = np.ascontiguousarray(W_qkv[:, C + c0:C + c0 + CSLICE])
        wv = np.ascontiguousarray(W_qkv[:, 2 * C + c0:2 * C + c0 + CSLICE])
        # spaced W_out: row g*128 + 32j + d = W_out[c0 + 16*(4g+j) + d]
        wo = np.zeros((G * 128, C), dtype=np.float32)
        for g in range(G):
            for j in range(4):
                src_r = c0 + HDIM * (4 * g + j)
                wo[g * 128 + 32 * j:g * 128 + 32 * j + HDIM, :] = \
                    W_out[src_r:src_r + HDIM, :]
        bq = np.zeros((G, 128), dtype=np.float32)
        bk = np.zeros((G, 128), dtype=np.float32)
        for g in range(G):
            for j in range(4):
                h = 8 * p + 4 * g + j
                bq[g, 32 * j:32 * j + HDIM] = b_qkv[HDIM * h:HDIM * (h + 1)]
                bk[g, 32 * j:32 * j + HDIM] = b_qkv[C + HDIM * h:C + HDIM * (h + 1)]
        bv = np.ascontiguousarray(b_qkv[2 * C + c0:2 * C + c0 + CSLICE]).reshape(1, CSLICE)
        bo = (b_out if p == 0 else np.zeros_like(b_out)).reshape(1, C)
        in_maps.append({
            "x": x2, "wq": wq, "wk": wk, "wv": wv, "wo": wo,
            "bq": bq, "bk": bk, "bv": bv.astype(np.float32),
            "bo": bo.astype(np.float32), "tri": tri,
        })
    return in_maps


def kernel(x, attn_mask, W_qkv, b_qkv, W_out, b_out):
    if "nc" not in _CACHE:
        nc = _build_nc()
        _legalize_waits(nc)   # sim-incompatible but required by walrus
        _CACHE["nc"] = nc
    nc = _CACHE["nc"]
    in_maps = _make_in_maps(x, W_qkv, b_qkv, W_out, b_out)
    res = run_bass_kernel_spmd(nc, in_maps, core_ids=list(range(NCORES)))
    y = np.zeros((T, C), dtype=np.float32)
    for r in res.results:
        y += r["y"].astype(np.float32)
    return y.reshape(1, T, C)


# revision 17
# speedup vs baseline: 1.0733x; 1.0733x over previous
"""Causal multi-head attention kernel for Trainium2 (8 NeuronCores).

Problem: x[1,2048,1024] -> qkv proj (W_qkv[1024,3072]) -> 64 heads of dim 16
         -> causal softmax attention -> out proj (W_out[1024,1024]).

Sharding: Megatron-style head parallelism. Each of the 8 cores owns 8 heads
(a 128-wide column slice of W_qkv per q/k/v and a 128-row slice of W_out),
computes a partial output projection, and the host sums the 8 partials
(the "all-reduce").

v2 pipeline notes (vs the phase-separated v1):
  * ScalarE exp over ~19M score elements is the critical resource (~160us),
    so the whole kernel is restructured as one ACT-paced stream:
    per query block qn: qkv(qn) -> attention tiles (kc-outer, a-inner) with
    qkv(qn+1) + outproj(qn-1) matmul units interleaved as PE filler inside
    the ACT-bound stretches, x-prefetch(qn+1) on DMA/GpSimd meanwhile.
  * ~1/3 of the full (non-diagonal) exp tiles are computed on the otherwise
    co-idle VectorE via a Schraudolph-style bf16 exp2 bit trick:
    bf16_bits(2^t) ~= round(128*t + 16256 - sigma); one tensor_scalar
    (mult+add, fp32 PSUM in -> int16 out) then bitcast int16->bf16.
    Error is ~3% on those p values only; with ~1/5 of all tiles tricked the
    final output error stays ~0.5%, well under the 2e-2 gate.
  * Diagonal tiles compute/exp/accumulate only the valid causal suffix
    (no dead-prefix memsets); the 128-wide diagonal stripe is tri-masked.
  * Softmax 1/rowsum via 2-pass Ln then Exp(scale=-1) (Square pass dropped:
    NaNs in never-read lanes are harmless; rowsum lanes are positive).
  * PSUM budget: 2x2-bank score sets + 2x1-bank PV accumulators +
    2x1-bank filler accumulators (qkv/outproj) = 8 banks.

Self-contained: hardcodes all shapes; host code only slices inputs per core
and sums the 8 partial outputs.
"""

import numpy as np
from contextlib import ExitStack

import ml_dtypes

import concourse.bass as bass
import concourse.tile as tile
from concourse import mybir
from concourse.bass_utils import run_bass_kernel_spmd

F32 = mybir.dt.float32
BF16 = mybir.dt.bfloat16
I16 = mybir.dt.int16
AF = mybir.ActivationFunctionType

T = 2048
C = 1024
HDIM = 16
NHEADS = 64
NCORES = 8
HPC = NHEADS // NCORES      # 8 heads per core
CSLICE = HPC * HDIM         # 128 channel slice per core
G = 2                       # head groups of 4 per core
NCH = C // 128              # 8 contraction chunks
NT = T // 128               # 16 token chunks of 128
NQ = T // 512               # 4 query blocks of 512

# Schraudolph bf16 exp2: bits = round(EXPQ_MUL * s + EXPQ_ADD) viewed as bf16
# approximates exp(0.25*s).  128*log2(e)*0.25 = 46.166...; 16256 = 127<<7.
EXPQ_MUL = 128.0 * 0.25 * 1.4426950408889634
EXPQ_ADD = 16256.0 - 5.5
TRICK_EVERY = 3            # every 3rd full tile -> DVE trick exp

_CACHE = {}


def _legalize_waits(nc):
    """This neuronxcc/walrus build encodes at most ONE sync-wait per
    instruction (two on EventSemaphore) — multi-wait sync_info dies in
    codegen with "Too many sync wait commands".  Hoist excess waits into
    standalone EventSemaphore instructions on the same engine immediately
    before the instruction (engine queues are in-order, so semantics are
    preserved)."""
    import bass_rust
    n = 0
    for f in nc.m.functions:
        for blk in f.blocks:
            out = []
            changed = False
            for inst in blk.instructions:
                si = inst.sync_info
                waits = list(si.on_wait) if si is not None and si.on_wait else []
                cap = 2 if isinstance(inst, mybir.InstEventSemaphore) else 1
                if len(waits) > cap:
                    extra, keep = waits[:-cap], waits[-cap:]
                    for i in range(0, len(extra), 2):
                        ev = mybir.InstEventSemaphore(
                            name=f"evwait-{n}", ins=[], outs=[])
                        n += 1
                        ev.engine = inst.engine
                        ev.sync_info = bass_rust.SyncInfo(
                            on_wait=extra[i:i + 2], on_update=[])
                        out.append(ev)
                    inst.sync_info = bass_rust.SyncInfo(
                        on_wait=keep,
                        on_update=list(si.on_update) if si.on_update else [])
                    changed = True
                out.append(inst)
            if changed:
                blk.instructions = out
    return n


def _build_nc():
    nc = bass.Bass()

    x_d = nc.declare_dram_parameter("x", [T, C], F32, isOutput=False)
    wq_d = nc.declare_dram_parameter("wq", [C, CSLICE], F32, isOutput=False)
    wk_d = nc.declare_dram_parameter("wk", [C, CSLICE], F32, isOutput=False)
    wv_d = nc.declare_dram_parameter("wv", [C, CSLICE], F32, isOutput=False)
    wo_d = nc.declare_dram_parameter("wo", [G * 128, C], F32, isOutput=False)
    bq_d = nc.declare_dram_parameter("bq", [G, 128], F32, isOutput=False)
    bk_d = nc.declare_dram_parameter("bk", [G, 128], F32, isOutput=False)
    bv_d = nc.declare_dram_parameter("bv", [1, CSLICE], F32, isOutput=False)
    bo_d = nc.declare_dram_parameter("bo", [1, C], F32, isOutput=False)
    tri_d = nc.declare_dram_parameter("tri", [128, 128], BF16, isOutput=False)
    y_d = nc.declare_dram_parameter("y", [T, C], F32, isOutput=True)

    with tile.TileContext(nc) as tc, ExitStack() as ctx:
        consts = ctx.enter_context(tc.tile_pool(name="consts", bufs=1))
        stage = ctx.enter_context(tc.tile_pool(name="stage", bufs=3))
        epool = ctx.enter_context(tc.tile_pool(name="epool", bufs=6))
        small = ctx.enter_context(tc.tile_pool(name="small", bufs=2))

        psco = ctx.enter_context(tc.tile_pool(name="psco", bufs=2, space="PSUM"))
        ppv = ctx.enter_context(tc.tile_pool(name="ppv", bufs=2, space="PSUM"))
        pfil = ctx.enter_context(tc.tile_pool(name="pfil", bufs=2, space="PSUM"))

        # ---- constants ----
        tri = consts.tile([128, 128], BF16)
        nc.sync.dma_start(out=tri, in_=tri_d[:, :])
        eps_sb = consts.tile([128, 1], F32)
        nc.vector.memset(eps_sb, 1e-30)
        bq_sb = consts.tile([128, G], F32)
        nc.sync.dma_start(out=bq_sb, in_=bq_d.rearrange("g p -> p g"))
        bk_sb = consts.tile([128, G], F32)
        nc.sync.dma_start(out=bk_sb, in_=bk_d.rearrange("g p -> p g"))
        # free-dim-varying biases must be physically replicated across
        # partitions (DVE operands need nonzero partition step)
        bv_sb = consts.tile([128, CSLICE], F32)
        nc.sync.dma_start(out=bv_sb, in_=bv_d[0:1, :].to_broadcast((128, CSLICE)))
        bo_sb = consts.tile([128, C], F32)
        nc.sync.dma_start(out=bo_sb, in_=bo_d[0:1, :].to_broadcast((128, C)))

        # ---- weights: load fp32, cast to bf16 stationaries ----
        # wq/wk spaced: per group g, chunk cc: [128c, 128] with head j's 16
        # cols at free offset 32j.
        wq_sb = consts.tile([128, G, NCH, 128], BF16)
        wk_sb = consts.tile([128, G, NCH, 128], BF16)
        nc.vector.memset(wq_sb, 0.0)
        nc.vector.memset(wk_sb, 0.0)
        wv_sb = consts.tile([128, NCH, CSLICE], BF16)
        wo_sb = consts.tile([128, G, C], BF16)
        def load_weights():
            wqf = stage.tile([128, NCH, CSLICE], F32, tag="wstage", name="wqf", bufs=1)
            nc.sync.dma_start(out=wqf, in_=wq_d.rearrange("(a p) w -> p a w", p=128))
            wkf = stage.tile([128, NCH, CSLICE], F32, tag="wstage2", name="wkf", bufs=1)
            nc.sync.dma_start(out=wkf, in_=wk_d.rearrange("(a p) w -> p a w", p=128))
            wvf = stage.tile([128, NCH, CSLICE], F32, tag="wstage3", name="wvf", bufs=1)
            nc.sync.dma_start(out=wvf, in_=wv_d.rearrange("(a p) w -> p a w", p=128))
            for g in range(G):
                for j in range(4):
                    h = 4 * g + j
                    nc.vector.tensor_copy(
                        wq_sb[:, g, :, 32 * j:32 * j + HDIM],
                        wqf[:, :, HDIM * h:HDIM * (h + 1)])
                    nc.vector.tensor_copy(
                        wk_sb[:, g, :, 32 * j:32 * j + HDIM],
                        wkf[:, :, HDIM * h:HDIM * (h + 1)])
            nc.vector.tensor_copy(wv_sb, wvf)
            for g in range(G):
                wof = stage.tile([128, C], F32, tag="wofull", name="wof", bufs=2)
                nc.sync.dma_start(out=wof, in_=wo_d[g * 128:(g + 1) * 128, :])
                nc.vector.tensor_copy(wo_sb[:, g, :], wof)

        # ---- persistent activations ----
        xT = consts.tile([128, NCH, T], BF16)   # xT[c, cc, t] = x[t, 128cc+c]
        qT = consts.tile([128, G, T], BF16)     # spaced: head j at part 32j
        kT = consts.tile([128, G, T], BF16)
        V = consts.tile([128, NT, HPC * 32], BF16)  # [t, tt, 8*32]: 16 dims +
        # rowsum-ones col + zero pad per head (packed PV writes 32 rows/head)
        vr = V.rearrange("p t (h e) -> p t h e", h=HPC)
        nc.vector.memset(vr[:, :, :, HDIM:32], 0.0)
        nc.vector.memset(vr[:, :, :, HDIM:HDIM + 1], 1.0)
        # group-spaced attn output: head j of group g at partitions
        # 32j..32j+15 of attnT[:, g, :]; rows 16..31 of each quadrant must be
        # ZERO (wo has zero rows there, but 0*garbage-NaN would poison).
        attnT = consts.tile([128, G, T], BF16)
        nc.vector.memset(attnT, 0.0)

        # ---- pipeline building blocks ----
        def x_tile(tt, on_act):
            xs = stage.tile([128, C], F32, tag="xload", name="xs", bufs=4)
            nc.sync.dma_start(out=xs, in_=x_d[tt * 128:(tt + 1) * 128, :])
            xb = stage.tile([128, C], BF16, tag="xcast", name="xb", bufs=4)
            if on_act:
                nc.scalar.activation(out=xb, in_=xs, func=AF.Copy)
            else:
                nc.vector.tensor_copy(xb, xs)
            nc.sync.dma_start_transpose(
                out=xT[:, :, tt * 128:(tt + 1) * 128], in_=xb)

        qk_open = {}

        def qk_half(g, qn, which, half):
            """One 4-chunk half of a q or k projection accumulation.
            half=0 opens the PSUM group; half=1 finishes it + bias."""
            w_sb, b_sb, dst = ((wq_sb, bq_sb, qT) if which == "q"
                               else (wk_sb, bk_sb, kT))
            if half == 0:
                qk_open[(which, g)] = pfil.tile(
                    [128, 512], F32, tag="fil", name="filps")
            ps_t = qk_open[(which, g)]
            for i in range(4):
                cc = 4 * half + i
                nc.tensor.matmul(
                    out=ps_t, lhsT=w_sb[:, g, cc, :],
                    rhs=xT[:, cc, qn * 512:(qn + 1) * 512],
                    start=(cc == 0), stop=(cc == NCH - 1),
                )
            if half == 1:
                del qk_open[(which, g)]
                nc.vector.tensor_scalar_add(
                    out=dst[:, g, qn * 512:(qn + 1) * 512], in0=ps_t,
                    scalar1=b_sb[:, g:g + 1],
                )

        def v_tile(tt):
            ps_t = pfil.tile([128, 512], F32, tag="fil", name="vps")
            ps = ps_t[:, 0:CSLICE]
            for cc in range(NCH):
                nc.tensor.matmul(
                    out=ps, lhsT=xT[:, cc, tt * 128:(tt + 1) * 128],
                    rhs=wv_sb[:, cc, :],
                    start=(cc == 0), stop=(cc == NCH - 1),
                )
            nc.vector.tensor_tensor(
                vr[:, tt, :, 0:HDIM], ps.rearrange("p (h e) -> p h e", h=HPC),
                bv_sb.rearrange("p (h e) -> p h e", h=HPC),
                mybir.AluOpType.add,
            )

        def outproj_unit(tt, nn):
            ps = pfil.tile([128, 512], F32, tag="fil", name="ops")
            for g in range(G):
                nc.tensor.matmul(
                    out=ps, lhsT=attnT[:, g, tt * 128:(tt + 1) * 128],
                    rhs=wo_sb[:, g, nn * 512:(nn + 1) * 512],
                    start=(g == 0), stop=(g == G - 1),
                )
            ys = stage.tile([128, 512], F32, tag="yout", name="ys")
            nc.vector.tensor_tensor(
                ys, ps, bo_sb[:, nn * 512:(nn + 1) * 512],
                mybir.AluOpType.add,
            )
            nc.sync.dma_start(
                out=y_d[tt * 128:(tt + 1) * 128, nn * 512:(nn + 1) * 512],
                in_=ys,
            )

        # filler scheduling: closures popped between attention tile groups
        fillers = []

        def pop_fillers(k):
            for _ in range(min(k, len(fillers))):
                fillers.pop(0)()

        trick_ctr = [0]

        def attn_group(g, qn):
            """Attention for 4 heads (group g) x 512 queries (block qn).
            Software-pipelined: scores for tile i+1 are emitted between
            exp(i) and PV(i) so ScalarE never waits on the PE queue."""
            pv = ppv.tile([128, 512], F32, tag="pv")
            nkc = 4 * qn + 4
            tiles = [(kc, a) for kc in range(nkc) for a in range(2)]
            ssets = {}

            def emit_S(idx):
                kc, a = tiles[idx]
                f0 = max(0, 128 * (kc - 4 * qn))
                sset = psco.tile([128, 1024], F32, tag="sset", name="sset")
                for jj in range(2):
                    j = 2 * a + jj
                    nc.tensor.matmul(
                        out=sset[:, 512 * jj + f0:512 * jj + 512],
                        lhsT=kT[32 * j:32 * j + HDIM, g, kc * 128:(kc + 1) * 128],
                        rhs=qT[32 * j:32 * j + HDIM, g, qn * 512 + f0:(qn + 1) * 512],
                        start=True, stop=True,
                        tile_position=(32 * j, 0),
                    )
                ssets[idx] = sset

            emit_S(0)
            for idx, (kc, a) in enumerate(tiles):
                jjj = kc - 4 * qn          # >=0: diagonal-straddling tile
                f0 = max(0, 128 * jjj)
                sset = ssets.pop(idx)
                et = epool.tile([128, 1024], BF16, tag="expT", name="et")
                er = et.rearrange("p (h q) -> p h q", h=2)
                sr = sset.rearrange("p (h q) -> p h q", h=2)
                if jjj >= 0:
                    # diagonal tile: exp only the valid suffix
                    nc.scalar.activation(
                        out=er[:, :, f0:512], in_=sr[:, :, f0:512],
                        func=AF.Exp, scale=0.25)
                else:
                    trick_ctr[0] += 1
                    if trick_ctr[0] % TRICK_EVERY == 0:
                        # VectorE Schraudolph bf16 exp2 bit trick
                        nc.vector.tensor_scalar(
                            out=et.bitcast(I16), in0=sset,
                            scalar1=EXPQ_MUL, scalar2=EXPQ_ADD,
                            op0=mybir.AluOpType.mult,
                            op1=mybir.AluOpType.add,
                        )
                    else:
                        nc.scalar.activation(
                            out=et, in_=sset, func=AF.Exp, scale=0.25)
                if idx + 1 < len(tiles):
                    emit_S(idx + 1)
                # one filler unit lands where PE would idle awaiting exp
                pop_fillers(1)
                if jjj >= 0:
                    # triangle-mask the diagonal stripe on (idle) GpSimd
                    nc.gpsimd.tensor_tensor(
                        er[:, :, f0:f0 + 128], er[:, :, f0:f0 + 128],
                        tri[:, None, :].to_broadcast((128, 2, 128)),
                        mybir.AluOpType.mult,
                    )
                for jj in range(2):
                    j = 2 * a + jj
                    h = 4 * g + j
                    nc.tensor.matmul(
                        out=pv[32 * j:32 * j + 32, f0:512],
                        lhsT=V[:, kc, 32 * h:32 * h + 32],
                        rhs=et[:, 512 * jj + f0:512 * jj + 512],
                        start=(kc == 0), stop=(kc == nkc - 1),
                        tile_position=(0, 32 * j),
                        # sim group tracker is partition-base blind;
                        # packed heads write disjoint partitions
                        skip_group_check=True,
                    )
            # normalize: 1/rowsum via exp(-ln(x+eps)); garbage lanes may go
            # NaN/inf but only the (positive) rowsum rows are ever read.
            ln_t = small.tile([128, 512], F32, tag="lnt")
            nc.scalar.activation(out=ln_t, in_=pv, func=AF.Ln, bias=eps_sb[:, 0:1])
            rec_t = small.tile([128, 512], F32, tag="rect")
            nc.scalar.activation(out=rec_t, in_=ln_t, func=AF.Exp, scale=-1.0)
            rec_rep = small.tile([128, 512], F32, tag="recrep")
            nc.vector.stream_shuffle(rec_rep, rec_t, [HDIM] * 32)
            for j in range(4):
                nc.vector.tensor_tensor(
                    attnT[32 * j:32 * j + HDIM, g, qn * 512:(qn + 1) * 512],
                    pv[32 * j:32 * j + HDIM, :],
                    rec_rep[32 * j:32 * j + HDIM, :],
                    mybir.AluOpType.mult,
                )

        # ---- emission: fused qn-major pipeline ----
        # x tiles 0-3: loads + ACT casts first in the queues, then the big
        # weight DMAs, then the xbar transposes (so nothing head-blocks).
        xb0 = []
        for tt in range(4):
            xs = stage.tile([128, C], F32, tag="xload", name="xs", bufs=4)
            nc.sync.dma_start(out=xs, in_=x_d[tt * 128:(tt + 1) * 128, :])
            xb = stage.tile([128, C], BF16, tag="xcast", name="xb", bufs=4)
            nc.scalar.activation(out=xb, in_=xs, func=AF.Copy)
            xb0.append(xb)
        load_weights()
        for tt in range(4):
            nc.sync.dma_start_transpose(
                out=xT[:, :, tt * 128:(tt + 1) * 128], in_=xb0[tt])
        # qkv for qn=0 emitted directly (nothing to hide it under yet)
        for g in range(G):
            for half in range(2):
                qk_half(g, 0, "q", half)
            for half in range(2):
                qk_half(g, 0, "k", half)
        for tt in range(4):
            v_tile(tt)

        for qn in range(NQ):
            # stage filler for the NEXT qn's qkv + previous qn's outproj;
            # x-prefetch DMAs issue now (DMA/DVE run independently).
            if qn + 1 < NQ:
                for tt in range(4 * qn + 4, 4 * qn + 8):
                    x_tile(tt, on_act=False)
                for g in range(G):
                    for half in range(2):
                        fillers.append(
                            lambda g=g, qn=qn, h=half: qk_half(g, qn + 1, "q", h))
                    for half in range(2):
                        fillers.append(
                            lambda g=g, qn=qn, h=half: qk_half(g, qn + 1, "k", h))
                for tt in range(4 * qn + 4, 4 * qn + 8):
                    fillers.append(lambda tt=tt: v_tile(tt))
            if qn > 0:
                for tt in range(4 * (qn - 1), 4 * qn):
                    for nn in range(2):
                        fillers.append(lambda tt=tt, nn=nn: outproj_unit(tt, nn))
            for g in range(G):
                attn_group(g, qn)
            # drain any leftover filler before moving on
            pop_fillers(len(fillers))
        for tt in range(12, 16):
            for nn in range(2):
                outproj_unit(tt, nn)
    return nc


def _make_in_maps(x, W_qkv, b_qkv, W_out, b_out):
    x2 = np.ascontiguousarray(np.asarray(x, dtype=np.float32).reshape(T, C))
    W_qkv = np.asarray(W_qkv, dtype=np.float32)
    b_qkv = np.asarray(b_qkv, dtype=np.float32)
    W_out = np.asarray(W_out, dtype=np.float32)
    b_out = np.asarray(b_out, dtype=np.float32)

    tri = np.zeros((128, 128), dtype=np.float32)
    for p in range(128):
        tri[p, p:] = 1.0
    tri = tri.astype(ml_dtypes.bfloat16)

    in_maps = []
    for p in range(NCORES):
        c0 = p * CSLICE
        wq = np.ascontiguousarray(W_qkv[:, c0:c0 + CSLICE])
        wk = np.ascontiguousarray(W_qkv[:, C + c0:C + c0 + CSLICE])
        wv = np.ascontiguousarray(W_qkv[:, 2 * C + c0:2 * C + c0 + CSLICE])
        # spaced W_out: row g*128 + 32j + d = W_out[c0 + 16*(4g+j) + d]
        wo = np.zeros((G * 128, C), dtype=np.float32)
        for g in range(G):
            for j in range(4):
                src_r = c0 + HDIM * (4 * g + j)
                wo[g * 128 + 32 * j:g * 128 + 32 * j + HDIM, :] = \
                    W_out[src_r:src_r + HDIM, :]
        bq = np.zeros((G, 128), dtype=np.float32)
        bk = np.zeros((G, 128), dtype=np.float32)
        for g in range(G):
            for j in range(4):
                h = 8 * p + 4 * g + j
                bq[g, 32 * j:32 * j + HDIM] = b_qkv[HDIM * h:HDIM * (h + 1)]
                bk[g, 32 * j:32 * j + HDIM] = b_qkv[C + HDIM * h:C + HDIM * (h + 1)]
        bv = np.ascontiguousarray(b_qkv[2 * C + c0:2 * C + c0 + CSLICE]).reshape(1, CSLICE)
        bo = (b_out if p == 0 else np.zeros_like(b_out)).reshape(1, C)
        in_maps.append({
            "x": x2, "wq": wq, "wk": wk, "wv": wv, "wo": wo,
            "bq": bq, "bk": bk, "bv": bv.astype(np.float32),
            "bo": bo.astype(np.float32), "tri": tri,
        })
    return in_maps


def kernel(x, attn_mask, W_qkv, b_qkv, W_out, b_out):
    if "nc" not in _CACHE:
        nc = _build_nc()
        _legalize_waits(nc)   # sim-incompatible but required by walrus
        _CACHE["nc"] = nc
    nc = _CACHE["nc"]
    in_maps = _make_in_maps(x, W_qkv, b_qkv, W_out, b_out)
    res = run_bass_kernel_spmd(nc, in_maps, core_ids=list(range(NCORES)))
    y = np.zeros((T, C), dtype=np.float32)
    for r in res.results:
        y += r["y"].astype(np.float32)
    return y.reshape(1, T, C)
